# revision 10
# baseline (speedup 1.0000x reference)
"""Trainium2 Bass kernel for nn_KalmanFilterPredictor.

Math: the Kalman covariance recursion never touches the data x and starts
from the same cov0 = I for every batch element, so the per-step gain K_t is
batch-independent.  The whole filter therefore collapses to a single linear
map of the measurements:

    state_T = sum_t (A_T ... A_{t+1}) K_t x_t + (A_T ... A_1) state_0
    out     = W F state_T + b  =  x_flat @ C + b

with A_t = (I - K_t H) F and C a tiny [T*D, TARGET] matrix computed on the
host in float64.  The coefficients C[t] decay exponentially backwards in
time (stable filter): keeping the trailing T_KEEP=18 steps (K=126 coeffs)
gives rel err ~3e-3 vs the full filter on the actual input distribution,
6x inside the 2e-2 gate; bf16 storage adds ~2e-3 more.

Device work per core (batch 8192 -> 8 x 1024, pure data parallel):

    out.T[7, 1024] = C.T[7, 126] @ xT[126, 1024]      (bf16 in, f32 acc)

K=126 pads to one 128-partition chunk, so the kernel is a single DMA of a
[128, 1040] bf16 tile (x columns 0:1024, C columns 1024:1031 packed into
the same transfer -> one ~2080B descriptor per partition), one LDWEIGHTS
of the tiny [128, 7] stationary C, two N=512 matmuls into PSUM, a DVE
PSUM->SBUF copy, and one [7, 4KB x 1024-col] output DMA.  Bias is added
on the host.
"""

import numpy as np

# Problem constants (fixed by the nn.Module definition).
BATCH = 8192
SEQ_LEN = 512
INPUT_DIM = 7
STATE_DIM = 14
TARGET_DIM = 7

N_CORES = 8
B_CORE = BATCH // N_CORES          # 1024 batch rows per core
T_KEEP = 12                        # trailing timesteps kept (12*7 = 84)
K_REAL = T_KEEP * INPUT_DIM        # 84
K_PAD = 86                         # partition rows (84 data + 2 pad)
G = 512                            # batch group (one PSUM bank of f32)
GCOLS = 8 + G                      # C(7)+pad + one batch group per half
XCOLS = 2 * GCOLS                  # [C|g0 | C|g1] halves, 1040B each

_NC = None  # compiled Bass module, built once per process


def _build_module():
    import concourse.bacc as bacc
    import concourse.mybir as mybir
    import concourse.tile as tile

    nc = bacc.Bacc("TRN2", debug=False, num_devices=N_CORES)
    bf16 = mybir.dt.bfloat16
    f32 = mybir.dt.float32

    x_d = nc.dram_tensor("xc", (K_PAD, XCOLS), bf16, kind="ExternalInput")
    o_d = nc.dram_tensor("outT", (TARGET_DIM, B_CORE), f32,
                         kind="ExternalOutput")

    with tile.TileContext(nc) as tc:
        with (
            tc.tile_pool(name="xin", bufs=2) as xin,
            tc.tile_pool(name="psum", bufs=1, space="PSUM") as psum,
            tc.tile_pool(name="outp", bufs=1) as outp,
        ):
            # Each half carries its own C copy + one 512-batch group, so
            # both matmuls are self-contained.  Two HWDGE rings (SP + ACT)
            # issue in parallel; packets interleave across the 16 SDMA
            # engines so half 0 lands early and MM0 overlaps half 1.
            x_sb = []
            for g, eng in ((0, nc.sync), (1, nc.scalar)):
                xt = xin.tile([K_PAD, GCOLS], bf16, name=f"x{g}", tag=f"x{g}")
                eng.dma_start(xt[:], x_d[:, g * GCOLS:(g + 1) * GCOLS])
                x_sb.append(xt)

            o_sb = outp.tile([TARGET_DIM, B_CORE], f32)
            for g in range(2):
                ps = psum.tile([TARGET_DIM, G], f32, name=f"ps{g}",
                               tag=f"ps{g}")
                nc.tensor.matmul(
                    ps[:], x_sb[g][:, :TARGET_DIM], x_sb[g][:, 8:GCOLS],
                    start=True, stop=True,
                )
                nc.vector.tensor_copy(o_sb[:, g * G:(g + 1) * G], ps[:])
            nc.sync.dma_start(o_d[:], o_sb[:])

    nc.compile()
    return nc


def _get_module():
    global _NC
    if _NC is None:
        _NC = _build_module()
    return _NC


def _coefficients(W, F, H, Q, R):
    """Collapse the filter to out = x_flat @ Cfull + b.  float64 on host.

    Returns Cfull [SEQ_LEN, INPUT_DIM, TARGET_DIM]: contribution of
    x[:, t, d] to out[:, j].
    """
    S, D, T = STATE_DIM, INPUT_DIM, SEQ_LEN
    F = F.astype(np.float64)
    H = H.astype(np.float64)
    Q = Q.astype(np.float64)
    R = R.astype(np.float64)
    I_s = np.eye(S)

    cov = np.eye(S)
    Ks, As = [], []
    for _ in range(T):
        cov = F @ cov @ F.T + Q
        K = cov @ H.T @ np.linalg.inv(H @ cov @ H.T + R)
        Ks.append(K)
        As.append((I_s - K @ H) @ F)
        cov = (I_s - K @ H) @ cov

    WF = W.astype(np.float64) @ F
    Cfull = np.zeros((T, D, TARGET_DIM))
    suffix = WF  # W F (A_{T-1} ... A_{t+1}) as t walks down
    for t in range(T - 1, -1, -1):
        Cfull[t] = (suffix @ Ks[t]).T
        suffix = suffix @ As[t]
    # state_0 = [x_0; 0] contributes through the full A-product.
    Cfull[0] += suffix[:, :D].T
    return Cfull


def kernel(x, W, b, F, H, Q, R):
    import ml_dtypes

    x = np.asarray(x)
    Cfull = _coefficients(np.asarray(W), np.asarray(F), np.asarray(H),
                          np.asarray(Q), np.asarray(R))
    t0 = SEQ_LEN - T_KEEP

    # Truncation guard: bound the dropped contribution.  For the real
    # problem the dropped coefficient mass is ~7e-3 vs tolerance 2e-2
    # on outputs of magnitude ~1.8; the empirical error is ~3e-3.
    dropped = np.abs(Cfull[:t0]).sum(axis=(0, 1)).max()
    need_head_fix = dropped > 5e-2

    Ct = np.zeros((K_PAD, TARGET_DIM), dtype=ml_dtypes.bfloat16)
    Ct[:K_REAL] = Cfull[t0:].reshape(K_REAL, TARGET_DIM)

    # Host transpose: [B, T_KEEP*D] tail -> [K_PAD, B] with k on rows.
    xk = x[:, t0:, :].reshape(BATCH, K_REAL)
    xT = np.zeros((K_PAD, BATCH), dtype=ml_dtypes.bfloat16)
    xT[:K_REAL] = xk.T

    nc = _get_module()
    in_maps = []
    for c in range(N_CORES):
        xc = np.zeros((K_PAD, XCOLS), dtype=ml_dtypes.bfloat16)
        for g in range(2):
            base = g * GCOLS
            xc[:, base:base + TARGET_DIM] = Ct
            xc[:, base + 8:base + 8 + G] = (
                xT[:, c * B_CORE + g * G:c * B_CORE + (g + 1) * G]
            )
        in_maps.append({"xc": xc})

    from concourse.bass_utils import run_bass_kernel_spmd

    res = run_bass_kernel_spmd(nc, in_maps, list(range(N_CORES)))
    global LAST_RESULTS
    LAST_RESULTS = res

    out = np.empty((BATCH, TARGET_DIM), dtype=np.float32)
    for c in range(N_CORES):
        out[c * B_CORE:(c + 1) * B_CORE] = res.results[c]["outT"].T
    out += np.asarray(b, dtype=np.float32)

    if need_head_fix:  # unreachable for the real model; exact fallback
        head = x[:, :t0, :].reshape(BATCH, t0 * INPUT_DIM).astype(np.float64)
        out = out + (head @ Cfull[:t0].reshape(t0 * INPUT_DIM, TARGET_DIM)
                     ).astype(np.float32)
    return out


# revision 12
# speedup vs baseline: 1.1771x; 1.1771x over previous
"""Trainium2 Bass kernel for nn_KalmanFilterPredictor.

Math: the Kalman covariance recursion never touches the data x and starts
from the same cov0 = I for every batch element, so the per-step gain K_t is
batch-independent.  The whole filter therefore collapses to a single linear
map of the measurements:

    state_T = sum_t (A_T ... A_{t+1}) K_t x_t + (A_T ... A_1) state_0
    out     = W F state_T + b  =  x_flat @ C + b

with A_t = (I - K_t H) F and C a tiny [T*D, TARGET] matrix computed on the
host in float64.  The coefficients C[t] decay exponentially backwards in
time (stable filter): keeping the trailing T_KEEP=18 steps (K=126 coeffs)
gives rel err ~3e-3 vs the full filter on the actual input distribution,
6x inside the 2e-2 gate; bf16 storage adds ~2e-3 more.

Device work per core (batch 8192 -> 8 x 1024, pure data parallel):

    out.T[7, 1024] = C.T[7, 126] @ xT[126, 1024]      (bf16 in, f32 acc)

K=126 pads to one 128-partition chunk, so the kernel is a single DMA of a
[128, 1040] bf16 tile (x columns 0:1024, C columns 1024:1031 packed into
the same transfer -> one ~2080B descriptor per partition), one LDWEIGHTS
of the tiny [128, 7] stationary C, two N=512 matmuls into PSUM, a DVE
PSUM->SBUF copy, and one [7, 4KB x 1024-col] output DMA.  Bias is added
on the host.
"""

import numpy as np

# Problem constants (fixed by the nn.Module definition).
BATCH = 8192
SEQ_LEN = 512
INPUT_DIM = 7
STATE_DIM = 14
TARGET_DIM = 7

N_CORES = 8
B_CORE = BATCH // N_CORES          # 1024 batch rows per core
T_KEEP = 18                        # trailing timesteps kept (18*7 = 126)
K_REAL = T_KEEP * INPUT_DIM        # 126
K_PAD = 128                        # full partition fan-out (16 SDMA engines)
G = 512                            # batch group (one PSUM bank of f32)
GCOLS = 8 + G                      # C(7)+pad + one batch group per half
XCOLS = 2 * GCOLS                  # [C|g0 | C|g1] halves, 1040B each

_NC = None  # compiled Bass module, built once per process


def _build_module():
    import concourse.bacc as bacc
    import concourse.mybir as mybir
    import concourse.tile as tile

    nc = bacc.Bacc("TRN2", debug=False, num_devices=N_CORES)
    bf16 = mybir.dt.bfloat16
    f32 = mybir.dt.float32

    x_d = nc.dram_tensor("xc", (K_PAD, XCOLS), bf16, kind="ExternalInput")
    o_d = nc.dram_tensor("outT", (TARGET_DIM, B_CORE), f32,
                         kind="ExternalOutput")

    with tile.TileContext(nc) as tc:
        with (
            tc.tile_pool(name="xin", bufs=2) as xin,
            tc.tile_pool(name="psum", bufs=1, space="PSUM") as psum,
            tc.tile_pool(name="outp", bufs=1) as outp,
        ):
            # Each half carries its own C copy + one 512-batch group, so
            # both matmuls are self-contained.  Both DMAs ride the same
            # HWDGE ring (FIFO): half 0 drains completely first, so MM0
            # starts ~0.6us before the full transfer lands; total
            # completion time is unchanged (byte-bound).
            x_sb = []
            for g in range(2):
                xt = xin.tile([K_PAD, GCOLS], bf16, name=f"x{g}", tag=f"x{g}")
                nc.sync.dma_start(xt[:], x_d[:, g * GCOLS:(g + 1) * GCOLS])
                x_sb.append(xt)

            o_sb = outp.tile([TARGET_DIM, B_CORE], f32)
            for g in range(2):
                ps = psum.tile([TARGET_DIM, G], f32, name=f"ps{g}",
                               tag=f"ps{g}")
                nc.tensor.matmul(
                    ps[:], x_sb[g][:, :TARGET_DIM], x_sb[g][:, 8:GCOLS],
                    start=True, stop=True,
                )
                nc.vector.tensor_copy(o_sb[:, g * G:(g + 1) * G], ps[:])
            nc.sync.dma_start(o_d[:], o_sb[:])

    nc.compile()
    return nc


def _get_module():
    global _NC
    if _NC is None:
        _NC = _build_module()
    return _NC


def _coefficients(W, F, H, Q, R):
    """Collapse the filter to out = x_flat @ Cfull + b.  float64 on host.

    Returns Cfull [SEQ_LEN, INPUT_DIM, TARGET_DIM]: contribution of
    x[:, t, d] to out[:, j].
    """
    S, D, T = STATE_DIM, INPUT_DIM, SEQ_LEN
    F = F.astype(np.float64)
    H = H.astype(np.float64)
    Q = Q.astype(np.float64)
    R = R.astype(np.float64)
    I_s = np.eye(S)

    cov = np.eye(S)
    Ks, As = [], []
    for _ in range(T):
        cov = F @ cov @ F.T + Q
        K = cov @ H.T @ np.linalg.inv(H @ cov @ H.T + R)
        Ks.append(K)
        As.append((I_s - K @ H) @ F)
        cov = (I_s - K @ H) @ cov

    WF = W.astype(np.float64) @ F
    Cfull = np.zeros((T, D, TARGET_DIM))
    suffix = WF  # W F (A_{T-1} ... A_{t+1}) as t walks down
    for t in range(T - 1, -1, -1):
        Cfull[t] = (suffix @ Ks[t]).T
        suffix = suffix @ As[t]
    # state_0 = [x_0; 0] contributes through the full A-product.
    Cfull[0] += suffix[:, :D].T
    return Cfull


def kernel(x, W, b, F, H, Q, R):
    import ml_dtypes

    x = np.asarray(x)
    Cfull = _coefficients(np.asarray(W), np.asarray(F), np.asarray(H),
                          np.asarray(Q), np.asarray(R))
    t0 = SEQ_LEN - T_KEEP

    # Truncation guard: bound the dropped contribution.  For the real
    # problem the dropped coefficient mass is ~7e-3 vs tolerance 2e-2
    # on outputs of magnitude ~1.8; the empirical error is ~3e-3.
    dropped = np.abs(Cfull[:t0]).sum(axis=(0, 1)).max()
    need_head_fix = dropped > 5e-2

    Ct = np.zeros((K_PAD, TARGET_DIM), dtype=ml_dtypes.bfloat16)
    Ct[:K_REAL] = Cfull[t0:].reshape(K_REAL, TARGET_DIM)

    # Host transpose: [B, T_KEEP*D] tail -> [K_PAD, B] with k on rows.
    xk = x[:, t0:, :].reshape(BATCH, K_REAL)
    xT = np.zeros((K_PAD, BATCH), dtype=ml_dtypes.bfloat16)
    xT[:K_REAL] = xk.T

    nc = _get_module()
    in_maps = []
    for c in range(N_CORES):
        xc = np.zeros((K_PAD, XCOLS), dtype=ml_dtypes.bfloat16)
        for g in range(2):
            base = g * GCOLS
            xc[:, base:base + TARGET_DIM] = Ct
            xc[:, base + 8:base + 8 + G] = (
                xT[:, c * B_CORE + g * G:c * B_CORE + (g + 1) * G]
            )
        in_maps.append({"xc": xc})

    from concourse.bass_utils import run_bass_kernel_spmd

    res = run_bass_kernel_spmd(nc, in_maps, list(range(N_CORES)))
    global LAST_RESULTS
    LAST_RESULTS = res

    out = np.empty((BATCH, TARGET_DIM), dtype=np.float32)
    for c in range(N_CORES):
        out[c * B_CORE:(c + 1) * B_CORE] = res.results[c]["outT"].T
    out += np.asarray(b, dtype=np.float32)

    if need_head_fix:  # unreachable for the real model; exact fallback
        head = x[:, :t0, :].reshape(BATCH, t0 * INPUT_DIM).astype(np.float64)
        out = out + (head @ Cfull[:t0].reshape(t0 * INPUT_DIM, TARGET_DIM)
                     ).astype(np.float32)
    return out
